# revision 55
# baseline (speedup 1.0000x reference)
"""Trainium2 Bass kernel for the cross-head MultiHeadAttention module.

Reference computation (per row r of x flattened to (N*L, E)):
    q = x @ Wq; k = x @ Wk; v = x @ Wv           (E = 1024, H = 16, D = 64)
    energy[r, i, j] = sum_d q[r,i,d] * k[r,j,d]  (cross-head, per position)
    attn = softmax(energy / 32, axis=j)
    out[r, i, :] = sum_j attn[r,i,j] * v[r,j,:]
    y = out.reshape(R, E) @ Wo + bo

Distribution: data-parallel over rows (N*L = 16384 -> 2048 rows/core x 8).

Per-core design (all big matmuls in bf16 on the PE array):
  *  Q/K projections run transposed (features on partitions, rows free).
     Their attention-layout rebuild (qd2b/kht2 block-diagonal slabs) is a
     pure partition shift, done as 4 direct SBUF->SBUF DMAs per tensor.
  *  V runs natural (rows on partitions) and round-trips through DRAM in
     a row-permuted (w, b, B, m, h) layout, so the readback into the
     block-diagonal vd slabs is 8 DMAs with 3-dim APs.  All relayout
     buffers ping-pong on pass parity.
  *  DMA instruction count is kept minimal because every dma_start costs
     ~650ns of *serial* sequencer (DIRECT2D) time; all issue from the SP
     sequencer, whose per-iteration stream is ordered by data readiness
     so no instruction head-of-line blocks a later-ready one.
  *  Energy: ONE matmul per row pair (pi, pi+RCp/2); softmax on dense
     psum banks; A@V one matmul per pair into double-buffered 2-bank
     psum tiles; Wo full-width accumulation; bias added on host; y is
     written back in bf16 (host casts to f32).
  *  Schedule: depth-2 software pipeline at chunk granularity, V
     projection FIRST so its round trip completes mid-iteration:
       projv(p)+readback | e0(p-1) | projq(p)+relayout | e1(p-1) |
       av0(p-1) | projk(p)+relayout | av1(p-1) | Wo(p-1)
     Same-shape matmuls stay contiguous on the tensor queue (mixing
     shapes per-instruction breaks PE pipelining, ~+70ns/matmul), and
     attention-phase inputs are always >= 1/3 iteration old.
  *  Memsets run on the idle Pool engine; psum->sbuf copies are spread
     V/S; startup loads are split so the first matmul starts ~10us in.
"""

import numpy as np
import ml_dtypes

import concourse.bass as bass
from concourse import bacc
import concourse.tile as tile
from concourse import mybir
from concourse.bass_utils import run_bass_kernel_spmd

F32 = mybir.dt.float32
BF16 = mybir.dt.bfloat16
AF = mybir.ActivationFunctionType
ALU = mybir.AluOpType
AX = mybir.AxisListType

E = 1024
H = 16
D = 64
NCORE = 8
RCMAX = 512


def build_nc(R, sizes):
    """Per-core kernel program: R rows total, pass sizes `sizes`."""
    assert sum(sizes) == R and all(s % 256 == 0 and s <= RCMAX for s in sizes)
    NP = len(sizes)
    starts = [sum(sizes[:i]) for i in range(NP)]

    nc = bacc.Bacc("TRN2", target_bir_lowering=False, debug=False)

    xt = nc.dram_tensor("xt", [E, R], BF16, kind="ExternalInput")
    wq = nc.dram_tensor("wq", [E, E], BF16, kind="ExternalInput")
    wk = nc.dram_tensor("wk", [E, E], BF16, kind="ExternalInput")
    wv = nc.dram_tensor("wv", [E, E], BF16, kind="ExternalInput")
    wo = nc.dram_tensor("wo", [E, E], BF16, kind="ExternalInput")
    yt = nc.dram_tensor("yt", [E, R], BF16, kind="ExternalOutput")

    with tile.TileContext(nc) as tc:
        with (
            tc.tile_pool(name="wpool", bufs=1) as wpool,      # persistent
            tc.tile_pool(name="xpool", bufs=2) as xpool,      # xt chunks
            tc.tile_pool(name="spool", bufs=1) as spool,      # q/k/v staging
            tc.tile_pool(name="apool", bufs=2) as apool,      # softmax temps
            tc.tile_pool(name="opool", bufs=1) as opool,      # oft2
            tc.tile_pool(name="ypool", bufs=2) as ypool,      # y staging
            tc.tile_pool(name="dram", bufs=2, space="DRAM") as dpool,
            tc.tile_pool(name="pproj", bufs=2, space="PSUM") as pproj,
            tc.tile_pool(name="pe", bufs=2, space="PSUM") as pe_pool,
            tc.tile_pool(name="pav", bufs=2, space="PSUM") as pav,
        ):
            # ---- persistent loads: xtc(0) + Wv + Wq split per chunk so
            # the first projections start early; Wk/Wo single-call.
            wq_sb = wpool.tile([128, 8, E], BF16, tag="wq")
            wk_sb = wpool.tile([128, 8, E], BF16, tag="wk")
            wv_sb = wpool.tile([128, 8, E], BF16, tag="wv")
            wo_sb = wpool.tile([128, 8, E], BF16, tag="wo")

            xtr = xt.rearrange("(c p) r -> p c r", p=128)
            xtc0 = xpool.tile([128, 8, RCMAX], BF16, tag="xtc")
            RC0 = sizes[0]
            # the very first V matmul needs only xtc c0 rows 0:128 and the
            # first half of wv c0 — split those off so it starts earliest
            nc.sync.dma_start(xtc0[:, 0, 0:128], xtr[:, 0, 0:128])
            wvr = wv.rearrange("(c p) e -> p c e", p=128)
            nc.sync.dma_start(wv_sb[:, 0, 0:512], wvr[:, 0, 0:512])
            nc.sync.dma_start(xtc0[:, 0, 128:RC0], xtr[:, 0, 128:RC0])
            nc.sync.dma_start(wv_sb[:, 0, 512:1024], wvr[:, 0, 512:1024])
            for c in range(1, 8):
                nc.sync.dma_start(xtc0[:, c, 0:RC0], xtr[:, c, 0:RC0])
                nc.sync.dma_start(wv_sb[:, c, :], wvr[:, c, :])
            wqr = wq.rearrange("(c p) e -> p c e", p=128)
            for c in range(8):
                nc.sync.dma_start(wq_sb[:, c, :], wqr[:, c, :])
            nc.sync.dma_start(
                wk_sb[:], wk.rearrange("(c p) e -> p c e", p=128))
            nc.sync.dma_start(
                wo_sb[:], wo.rearrange("(c p) e -> p c e", p=128))

            # block-diagonal operand tiles (max size; per-pass slices):
            # zero blocks are memset once (on the idle Pool engine) and
            # never rewritten.  Ping-pong on pass parity.
            qd2bs, kht2s, vds = [], [], []
            for pp in range(2):
                qd2b_ = wpool.tile([128, 32, RCMAX // 2], BF16,
                                   tag=f"qd2b{pp}", name=f"qd2b{pp}")
                nc.gpsimd.memset(qd2b_[0:64, 16:32, :], 0.0)
                nc.gpsimd.memset(qd2b_[64:128, 0:16, :], 0.0)
                qd2bs.append(qd2b_)
                kht2_ = wpool.tile([128, 16, RCMAX // 2], BF16,
                                   tag=f"kht2{pp}", name=f"kht2{pp}")
                kht2s.append(kht2_)
                vd_ = wpool.tile([128, (RCMAX // 256) * 32, 128], BF16,
                                 tag=f"vd{pp}", name=f"vd{pp}")
                nc.gpsimd.memset(vd_[:], 0.0)
                vds.append(vd_)

            oft2 = opool.tile([128, 8, RCMAX], BF16, tag="oft2")

            def proj_tr(name, w_sb, xtc, RC):
                """Transposed projection (features on partitions)."""
                stg = spool.tile([128, 8, RCMAX], BF16, tag=f"stg_{name}")
                for et in range(8):
                    ps = pproj.tile([128, RCMAX], F32, tag="proj")
                    for c in range(8):
                        nc.tensor.matmul(
                            ps[:, 0:RC],
                            w_sb[:, c, et * 128:(et + 1) * 128],
                            xtc[:, c, 0:RC],
                            start=(c == 0),
                            stop=(c == 7),
                        )
                    if et % 3 < 2:
                        nc.vector.tensor_copy(stg[:, et, 0:RC], ps[:, 0:RC])
                    else:
                        nc.scalar.copy(stg[:, et, 0:RC], ps[:, 0:RC])
                return stg

            def relayout_q(it, stg, RC):
                """stg_q -> qd2b[it%2] via 4 partition-shift SBUF DMAs.

                qd2b[64h'+d, 16h'+q, pi] = q^T[d, head q, row PH*h'+pi]
                and stg[64(q%2)+d, q//2, r] = q^T[d, head q, row r], so
                each (h', q%2) group of 8 heads is one strided DMA.
                """
                PH = RC // 2
                qd2b_w = qd2bs[it % 2]
                for hp in range(2):
                    for hs in range(2):
                        nc.sync.dma_start(
                            qd2b_w[64 * hp:64 * hp + 64,
                                   16 * hp + hs:16 * hp + 16:2, 0:PH],
                            stg[64 * hs:64 * hs + 64, :,
                                PH * hp:PH * hp + PH],
                        )

            def relayout_k(it, stg, RC):
                """stg_k -> kht2[it%2]: kht2[64h'+d, q, pi] = k^T[d, q,
                row PH*h'+pi]."""
                PH = RC // 2
                kht2_w = kht2s[it % 2]
                for hp in range(2):
                    for hs in range(2):
                        nc.sync.dma_start(
                            kht2_w[64 * hp:64 * hp + 64, hs:16:2, 0:PH],
                            stg[64 * hs:64 * hs + 64, :,
                                PH * hp:PH * hp + PH],
                        )

            def proj_v(it, xtc, RC):
                """Natural (row-major) V projection, staged to DRAM with
                rows permuted to (w, b, B, m, h) order so the vd readback
                needs only 8 DMA instructions.  Chunk rc_ holds the rows
                with (h, B) = divmod(rc_, NBK)."""
                NBK = RC // 256
                vstg = spool.tile([128, RCMAX // 128, E], BF16, tag="stg_v")
                v3d = dpool.tile([2, 4, RCMAX // 256, 16, 2, E], BF16,
                                 tag="dram_v")
                for rc_ in range(RC // 128):
                    for h2 in range(2):
                        ps = pproj.tile([128, RCMAX], F32, tag="proj")
                        for c in range(8):
                            nc.tensor.matmul(
                                ps[:, 0:512],
                                xtc[:, c, rc_ * 128:(rc_ + 1) * 128],
                                wv_sb[:, c, h2 * 512:(h2 + 1) * 512],
                                start=(c == 0),
                                stop=(c == 7),
                            )
                        if h2 == 0:
                            nc.vector.tensor_copy(vstg[:, rc_, 0:512],
                                                  ps[:, 0:512])
                        else:
                            nc.scalar.copy(vstg[:, rc_, 512:1024],
                                           ps[:, 0:512])
                    h, B = divmod(rc_, NBK)
                    for w in range(2):
                        for b in range(4):
                            nc.sync.dma_start(
                                v3d[w, b, B, :, h, :],
                                vstg[64 * w + 16 * b:64 * w + 16 * b + 16,
                                     rc_, :],
                            )
                return v3d

            def vd_readback(it, v3d, RC):
                """DRAM -> vd[it%2] block-diagonal V slabs: one DMA per
                (row-half w2, quarter b), 3-dim APs on both sides."""
                NBK = RC // 256
                vd_w = vds[it % 2]
                for w2 in range(2):
                    for b in range(4):
                        nc.sync.dma_start(
                            vd_w[32 * b + 16 * w2:32 * b + 16 * w2 + 16,
                                 0:NBK * 32, 64 * w2:64 * w2 + 64],
                            v3d[w2, b, 0:NBK].rearrange(
                                "B m h (j d) -> j (B m h) d", j=16, d=64),
                        )

            def energy_softmax(B, par):
                """One dense 256-row energy bank + its softmax; returns att."""
                qd2b, kht2 = qd2bs[par], kht2s[par]
                ep = pe_pool.tile([128, 32, 16], F32, tag="ep",
                                  name=f"ep_{B}_{par}")
                for lam in range(128):
                    pi = 128 * B + lam
                    b, s = (lam // 16) % 4, 2 * (lam % 16) + lam // 64
                    nc.tensor.matmul(
                        ep[32 * b:32 * b + 32, s, :],
                        qd2b[:, :, pi],
                        kht2[:, :, pi],
                        start=True,
                        stop=True,
                        tile_position=(0, 32 * b),
                    )
                ex = apool.tile([128, 32, 16], F32, tag="ex",
                                name=f"ex_{B}_{par}")
                nc.scalar.activation(ex[:], ep[:], AF.Exp)
                sm = apool.tile([128, 32], F32, tag="sm", name=f"sm_{B}_{par}")
                nc.vector.reduce_sum(sm[:], ex[:], axis=AX.X)
                rcp = apool.tile([128, 32], F32, tag="rcp",
                                 name=f"rcp_{B}_{par}")
                nc.vector.reciprocal(rcp[:], sm[:])
                at = apool.tile([128, 32, 16], BF16, tag="at",
                                name=f"at_{B}_{par}")
                nc.vector.tensor_tensor(
                    at[:], ex[:],
                    rcp[:, :, None].to_broadcast([128, 32, 16]),
                    ALU.mult,
                )
                att = apool.tile([128, 512], BF16, tag="att",
                                 name=f"att_{B}_{par}")
                nc.vector.transpose(att[:], at[:].rearrange("p a b -> p (a b)"))
                return att

            def av_extract(B, att, par, RC):
                """A@V for bank B: two 2-bank psum tiles (b-halves) + 8
                merged extraction copies."""
                NBK = RC // 256
                vd = vds[par]
                dstx = oft2[:, :, 0:RC].rearrange(
                    "p g (h Bk wc) -> p g h Bk wc", h=2, Bk=NBK)
                for bh in range(2):
                    avp = pav.tile([128, 2, 32, 16], F32, tag="avp",
                                   name=f"avp_{B}_{bh}_{par}")
                    for b2 in range(2):
                        b = 2 * bh + b2
                        for t in range(32):
                            nc.tensor.matmul(
                                avp[:, b2, t, :],
                                vd[32 * b:32 * b + 32, 32 * B + t, :],
                                att[32 * b:32 * b + 32, 16 * t:16 * t + 16],
                                start=True,
                                stop=True,
                                tile_position=(32 * b, 0),
                            )
                    # avp[64w+d, b2, 2m+rho, q] -> oft2[64(q%2)+d, q//2,
                    #   (RC/2)*rho + 128B + 64w + 32bh + 16b2 + m]
                    srcx = avp[:].rearrange(
                        "p b (m r) (g s) -> p g r (b m) s", r=2, s=2)
                    for w in range(2):
                        for sg in range(2):
                            srcc = srcx[64 * w:64 * w + 64, :, :, :, sg]
                            dst = dstx[64 * sg:64 * sg + 64, :, :, B,
                                       64 * w + 32 * bh:64 * w + 32 * bh + 32]
                            if (w + sg + B + bh) % 2 == 0:
                                nc.vector.tensor_copy(dst, srcc)
                            else:
                                nc.scalar.copy(dst, srcc)

            def wo_out(p, RC, r0):
                """y^T = Wo^T-chunks @ oft2, DMA out (bias added on host)."""
                for c in range(8):
                    # rotates through the ep buffers (attention is done
                    # with them by now) -> double-buffered Wo psum at no
                    # extra bank cost
                    ytp = pe_pool.tile([128, RCMAX], F32, tag="ep")
                    for g in range(8):
                        nc.tensor.matmul(
                            ytp[:, 0:RC],
                            wo_sb[:, g, 128 * c:128 * c + 128],
                            oft2[:, g, 0:RC],
                            start=(g == 0),
                            stop=(g == 7),
                        )
                    ys = ypool.tile([128, RCMAX], BF16, tag="ys")
                    ytr = yt.rearrange("(t q) r -> q t r", q=128)
                    if p == NP - 1:
                        # final pass: split copy+DMA in halves so the last
                        # output DMA overlaps the closing Wo matmuls
                        for hf in range(2):
                            sl = slice(hf * (RC // 2), (hf + 1) * (RC // 2))
                            if (c + hf) % 2 == 0:
                                nc.vector.tensor_copy(ys[:, sl], ytp[:, sl])
                            else:
                                nc.scalar.copy(ys[:, sl], ytp[:, sl])
                            nc.sync.dma_start(
                                ytr[:, c, r0 + sl.start:r0 + sl.stop],
                                ys[:, sl])
                    else:
                        if c % 2 == 0:
                            nc.vector.tensor_copy(ys[:, 0:RC], ytp[:, 0:RC])
                        else:
                            nc.scalar.copy(ys[:, 0:RC], ytp[:, 0:RC])
                        nc.sync.dma_start(ytr[:, c, r0:r0 + RC], ys[:, 0:RC])

            # Depth-2 software pipeline: attention for pass p runs during
            # iteration p+1, interleaved with pass p+1's projections at
            # chunk granularity on the tensor queue.
            xtcs = {0: xtc0}
            for it in range(NP + 1):
                do_proj = it < NP
                do_att = it >= 1
                p = it - 1
                if do_proj:
                    RCi = sizes[it]
                    xtc = xtcs.pop(it)
                    # prefetch next pass's x chunk a full iteration ahead
                    if it + 1 < NP:
                        r0n, RCn = starts[it + 1], sizes[it + 1]
                        xn = xpool.tile([128, 8, RCMAX], BF16, tag="xtc")
                        nc.sync.dma_start(xn[:, :, 0:RCn],
                                          xtr[:, :, r0n:r0n + RCn])
                        xtcs[it + 1] = xn
                if do_att:
                    RCp, NBKp = sizes[p], sizes[p] // 256

                # --- interleaved tensor-queue schedule (V first so its
                # DRAM round trip completes mid-iteration) ---
                if do_proj:
                    v3d = proj_v(it, xtc, RCi)
                    vd_readback(it, v3d, RCi)
                if do_att:
                    att0 = energy_softmax(0, p % 2)
                if do_proj:
                    q_stg = proj_tr("q", wq_sb, xtc, RCi)
                    relayout_q(it, q_stg, RCi)
                if do_att:
                    att1 = energy_softmax(1, p % 2) if NBKp > 1 else None
                    av_extract(0, att0, p % 2, RCp)
                if do_proj:
                    k_stg = proj_tr("k", wk_sb, xtc, RCi)
                    relayout_k(it, k_stg, RCi)
                if do_att:
                    if att1 is not None:
                        av_extract(1, att1, p % 2, RCp)
                    wo_out(p, RCp, starts[p])

    nc.finalize()
    return nc


_CACHE = {}


def _get_nc(R, sizes):
    key = (R, tuple(sizes))
    if key not in _CACHE:
        _CACHE[key] = build_nc(R, sizes)
    return _CACHE[key]


def _pass_sizes(R):
    """Even passes of 512 (uneven tails measured slower)."""
    assert R % 256 == 0
    return [512] * (R // 512) + ([256] if R % 512 else [])


def run_cores(x2d, Wq, Wk, Wv, Wo, bo_v, R=None, cores=None, **run_kwargs):
    """x2d: (ROWS, E) fp32.  Returns (ROWS, E) fp32."""
    ROWS = x2d.shape[0]
    if cores is None:
        cores = list(range(NCORE))
    n = len(cores)
    if R is None:
        R = ROWS // n
    assert R * n == ROWS
    nc = _get_nc(R, _pass_sizes(R))

    bf = ml_dtypes.bfloat16
    scale = 1.0 / np.sqrt(np.sqrt(float(E)))  # fold E**-0.5 into both Wq, Wk
    wq_b = (Wq.astype(np.float64) * scale).astype(bf)
    wk_b = (Wk.astype(np.float64) * scale).astype(bf)
    wv_b = Wv.astype(bf)
    wo_b = Wo.astype(bf)
    bo_f = bo_v.reshape(1, E).astype(np.float32)

    in_maps = []
    for ci in range(n):
        xs = x2d[ci * R:(ci + 1) * R].T  # (E, R)
        in_maps.append({
            "xt": np.ascontiguousarray(xs).astype(bf),
            "wq": wq_b, "wk": wk_b, "wv": wv_b, "wo": wo_b,
        })
    res = run_bass_kernel_spmd(nc, in_maps, core_ids=cores, **run_kwargs)
    out = np.empty((ROWS, E), dtype=np.float32)
    for ci in range(n):
        ytd = res.results[ci]["yt"]  # (E, R) bf16, natural row order
        out[ci * R:(ci + 1) * R] = ytd.T.astype(np.float32) + bo_f
    if run_kwargs.get("trace"):
        return out, res
    return out


def kernel(x, Wq, Wk, Wv, Wo, bo):
    x = np.asarray(x, dtype=np.float32)
    N, L, _ = x.shape
    y = run_cores(
        x.reshape(N * L, E),
        np.asarray(Wq, np.float32), np.asarray(Wk, np.float32),
        np.asarray(Wv, np.float32), np.asarray(Wo, np.float32),
        np.asarray(bo, np.float32),
    )
    return y.reshape(N, L, E)
